# revision 24
# baseline (speedup 1.0000x reference)
"""DeepseekV3 attention (B=1, S=2048, D=2048, H=16, KV=4) on 8 trn2 cores.

Sharding: token-modulo-8 split. Core c owns query tokens {t : t % 8 == c}
(256 each) — causal attention work is identical on every core, so one SPMD
program serves all 8 cores with per-core DATA (host-sliced hidden columns,
cos/sin slices, causal band masks) carrying the differences.

All matmul operands are bf16 (fp32r runs as fp32_mode=HIGH at 4 cyc/row;
bf16 is 1 cyc/row and halves DMA). Attention processes head PAIRS in single
wide matmuls (rope q zero-padded into complementary partition halves so one
128-contraction matmul serves both heads), skips fully-masked query columns
(block kb only needs cols >= 16*kb), and applies the causal mask only on the
16-column diagonal band (identical [128,16] mask for every kb).

Scheduling: RMS scales are applied to q_a / c_kv ONCE (in place) so the
b-projection evacuations have no cross-engine gating; the kv token chunks
are software-pipelined (chunk n's b-projections run after chunk n+1's
a-projection, hiding the rms-chain latency); reciprocals run full-width on
DRAM-transposed [128, n] layouts; softmax normalization happens per head
pair, overlapped with the next pair's attention, folded into the AV psum
evacuation; bulk weight streams ride the otherwise-idle GpSimd DMA queue
while latency-critical small DMAs keep the Sync queue.
"""
import math
import sys
import types

import numpy as np
from ml_dtypes import bfloat16

# ---------------------------------------------------------------------------
# Container compat: this walrus build rejects instructions carrying more than
# one sync-wait command. Patch Tile to (a) split multi-wait instructions into
# single-wait NoOps on the same engine, (b) hoist the end-of-kernel drain's
# waits onto single-wait NOPs. Also register the NTFF profile hook (the
# image's antenv lacks axon_hooks) so trace=True works for profiling.
# ---------------------------------------------------------------------------
import concourse.bass as bass
import concourse.mybir as mybir
import concourse.tile as tile
from concourse.bass_utils import run_bass_kernel_spmd
from concourse.tile import ScopedClock
from bass_rust import VectorClock

N_PROCS = len(VectorClock())
_PATCHED = False


def _install_ntff_hook():
    if 'antenv.axon_hooks' in sys.modules:
        return
    m = types.ModuleType('antenv.axon_hooks')
    holder = [None]
    m.set_axon_ntff_profile_hook = lambda h: holder.__setitem__(0, h)
    m.get_axon_ntff_profile_hook = lambda: holder[0]
    sys.modules['antenv.axon_hooks'] = m
    try:
        from trn_agent_boot.trn_boot import _ntff_profile_via_ctypes
        m.set_axon_ntff_profile_hook(
            _ntff_profile_via_ctypes('/opt/axon/libaxon_pjrt.so'))
    except Exception:
        pass


def _patched_drain_and_barrier(self, tick_clock, wait_clock):
    gc = tick_clock.global_clock
    for p in range(N_PROCS):
        if gc[p] == 0:
            continue
        single = VectorClock([gc[q] if q == p else 0 for q in range(N_PROCS)])
        nop_inst = self.nc.sync.nop(nofuse=True)
        wait_clock.add_sem_waits(nop_inst.ins, ScopedClock({None: single}))
    self.nc.sync.drain()
    self.nc.all_engine_barrier()
    popped = self.nc._tile_sem_poison_stack.pop()
    assert popped is self._sem_poison
    self.nc.clear_and_free_semaphores(list(self.sems.allocated().values()))
    self.nc.all_engine_barrier()


def _make_split_lower(orig):
    def _split_multi_waits(self, ordered):
        nc = self.nc
        for bb_name, insts in ordered.items():
            out = []
            for inst in insts:
                si = inst.sync_info
                waits = list(si.on_wait) if si is not None else []
                if len(waits) > 1:
                    for w in waits[:-1]:
                        nop = mybir.InstNoOp(
                            name=f"{inst.name}-waitsplit-{nc.next_id()}",
                            engine=inst.engine,
                            sync_info=mybir.SyncInfo(on_wait=[w], on_update=[]),
                        )
                        nc.register_instruction(nop)
                        out.append(nop)
                    inst.sync_info = mybir.SyncInfo(
                        on_wait=[waits[-1]], on_update=list(si.on_update))
                out.append(inst)
            ordered[bb_name] = out
        return orig(self, ordered)
    return _split_multi_waits


def _install_patches():
    global _PATCHED
    _install_ntff_hook()
    if _PATCHED:
        return
    tile.TileContext._drain_and_barrier = _patched_drain_and_barrier
    tile.TileContext._lower_ordered_insts = _make_split_lower(
        tile.TileContext._lower_ordered_insts)
    _PATCHED = True


_install_patches()

# ---------------------------------------------------------------------------
# Problem constants (hardcoded per the spec).
# ---------------------------------------------------------------------------
S = 2048
D = 2048
H = 16
KV = 4
GROUPS = H // KV
DN = 128          # d_nope
DR = 64           # d_rope
DQK = DN + DR     # 192
DV = 128
QR = 1536         # q rank
KVR = 512         # kv rank
EPS = 1e-6
NC_ = 8           # cores
TPC = S // NC_    # 256 tokens per core
NB = S // 128     # 16 k-subtiles
NPAIR = H // 2    # 8 head pairs
SCALE = 1.0 / math.sqrt(DQK)
NEG = -1e30

F32 = mybir.dt.float32
BF16 = mybir.dt.bfloat16
AF = mybir.ActivationFunctionType

_BUILT = None     # cached (nc,) so repeat kernel() calls skip rebuild
LAST_RESULTS = None  # BassKernelResults stash for test.py


def _build():
    nc = bass.Bass()

    # ---- DRAM I/O (identical declaration on all cores; data differs) ----
    hT = nc.dram_tensor("hT", [D, S], BF16, kind="ExternalInput")
    hTq = nc.dram_tensor("hTq", [D, TPC], BF16, kind="ExternalInput")
    wqa = nc.dram_tensor("wqa", [D, QR], BF16, kind="ExternalInput")
    wqbn = nc.dram_tensor("wqbn", [QR, H * DN], BF16, kind="ExternalInput")
    wqbr = nc.dram_tensor("wqbr", [QR, H * DR], BF16, kind="ExternalInput")
    wkva = nc.dram_tensor("wkva", [D, KVR + DR], BF16, kind="ExternalInput")
    wkvbk = nc.dram_tensor("wkvbk", [KVR, KV * DN], BF16, kind="ExternalInput")
    wkvbv = nc.dram_tensor("wkvbv", [KVR, KV * DV], BF16, kind="ExternalInput")
    wo_t = nc.dram_tensor("wo", [H * DV, D], BF16, kind="ExternalInput")
    cossinT = nc.dram_tensor("cossinT", [2 * DR, S], BF16,
                             kind="ExternalInput")
    cosq2 = nc.dram_tensor("cosq2", [2 * DR, TPC], BF16, kind="ExternalInput")
    sinq2 = nc.dram_tensor("sinq2", [2 * DR, TPC], BF16, kind="ExternalInput")
    bmask = nc.dram_tensor("bmask", [128, 2 * 16], F32, kind="ExternalInput")
    out = nc.dram_tensor("out", [TPC, D], F32, kind="ExternalOutput")
    # scratch for free->partition broadcasts / transposed reciprocals
    scr_k = nc.dram_tensor("scr_k", [1, S], F32, kind="Internal")
    scr_k2 = nc.dram_tensor("scr_k2", [1, S], BF16, kind="Internal")
    scr_q = nc.dram_tensor("scr_q", [1, TPC], F32, kind="Internal")
    scr_q2 = nc.dram_tensor("scr_q2", [1, TPC], BF16, kind="Internal")
    scr_r = nc.dram_tensor("scr_r", [1, NPAIR * 2 * TPC], F32, kind="Internal")
    scr_r2 = nc.dram_tensor("scr_r2", [1, NPAIR * 2 * TPC], F32,
                            kind="Internal")

    def bcast_src(dram, off, ncols):
        # element off.. of the flat DRAM vector, broadcast to 128 partitions
        ap = dram[:]
        return bass.AP(tensor=ap.tensor, offset=ap.offset + off,
                       ap=[[0, 128], [1, ncols]])

    def transpose_ap(dram, off, nfree):
        # [128, nfree] view of dram[off .. off+128*nfree) with partition
        # stride 1 (transposed load/store for 1-row vectors)
        ap = dram[:]
        return bass.AP(tensor=ap.tensor, offset=ap.offset + off,
                       ap=[[1, 128], [128, nfree]])

    with tile.TileContext(nc) as tc:
        with (
            tc.tile_pool(name="persist", bufs=1) as P,   # attention-lived
            tc.tile_pool(name="wstream", bufs=3) as WS,  # streamed weights
            tc.tile_pool(name="ppool", bufs=6) as PP,    # p tiles (bf16)
        ):
            ones_b = P.tile([128, 1], BF16, name="ones_b")
            nc.vector.memset(ones_b[:], 1.0)
            eps_sb = P.tile([1, 1], F32, name="eps_sb")
            nc.vector.memset(eps_sb[:], EPS)
            # band mask [k, head, i]: 0 if k <= 8i + c else NEG (same for
            # every key block kb; applies to query cols 16kb..16kb+16)
            bmask_sb = P.tile([128, 2, 16], F32, name="bmask_sb")
            nc.sync.dma_start(bmask_sb[:], bmask[:, :].rearrange(
                "k (h i) -> k h i", h=2))

            # attention-lived products, head-pair packed
            qn_pair = [P.tile([128, 2, TPC], BF16, name=f"qnp{j}")
                       for j in range(NPAIR)]
            qr_pair = [P.tile([128, 2, TPC], BF16, name=f"qrp{j}")
                       for j in range(NPAIR)]
            for j in range(NPAIR):
                nc.vector.memset(qr_pair[j][:], 0.0)
            knopeT = [P.tile([128, S], BF16, name=f"knopeT{h}")
                      for h in range(KV)]
            v_sb = [P.tile([128, KV * DV], BF16, name=f"v{m}")
                    for m in range(16)]
            # k_rot^T duplicated in both partition halves so the rope scores
            # matmul can serve both heads of a pair in one 128-contraction
            krot2 = P.tile([128, S], BF16, name="krot2")

            # ========================= KV window =========================
            # replicated over all 2048 tokens, 4 chunks of 512, software
            # pipelined: chunk n's b-projections are issued after chunk
            # n+1's a-projection so the rms chain latency hides under PE
            # work. ckv is rms-scaled IN PLACE once per chunk, so the
            # b-projection evacuations are plain copies with no gating.
            with (
                tc.tile_pool(name="kvwin", bufs=1) as KW,
                tc.tile_pool(name="kvch", bufs=2) as KC,
                tc.tile_pool(name="ksc", bufs=2) as KS,
                tc.tile_pool(name="kps", bufs=1, space="PSUM") as PSB,
            ):
                wkva_sb = [KW.tile([128, KVR + DR], BF16, name=f"wkva{k}")
                           for k in range(16)]
                for k in range(16):
                    nc.gpsimd.dma_start(wkva_sb[k][:],
                                        wkva[k * 128:(k + 1) * 128, :])
                wkvbk_sb = [KW.tile([128, KV * DN], BF16, name=f"wkvbk{k}")
                            for k in range(4)]
                wkvbv_sb = [KW.tile([128, KV * DV], BF16, name=f"wkvbv{k}")
                            for k in range(4)]
                for k in range(4):
                    nc.gpsimd.dma_start(wkvbk_sb[k][:],
                                        wkvbk[k * 128:(k + 1) * 128, :])
                    nc.gpsimd.dma_start(wkvbv_sb[k][:],
                                        wkvbv[k * 128:(k + 1) * 128, :])

                m_sizes = [128, 128, 128, 128, 64]
                ckv_n = [None] * 4   # per-chunk scaled ckv tiles

                def kv_a(n):
                    ncols = slice(n * 512, (n + 1) * 512)
                    ckv = [KC.tile([m_sizes[m], 512], BF16, name=f"ckv{m}",
                                   tag=f"ckv{m}") for m in range(5)]
                    ckv_n[n] = ckv
                    pss = [PSB.tile([m_sizes[m], 512], F32, name=f"ps_kva{m}",
                                    tag=f"ps_kva{m}") for m in range(5)]
                    for k in range(16):
                        hch = WS.tile([128, 512], BF16, name="hch", tag="hch")
                        nc.gpsimd.dma_start(hch[:],
                                            hT[k * 128:(k + 1) * 128, ncols])
                        for m in range(5):
                            nc.tensor.matmul(
                                pss[m][:],
                                wkva_sb[k][:, m * 128: m * 128 + m_sizes[m]],
                                hch[:], start=(k == 0), stop=(k == 15))
                    for m in range(5):
                        nc.vector.tensor_copy(ckv[m][:], pss[m][:])

                    # RoPE on k_rot (raw; no rms on the rope part)
                    cos_t = KS.tile([64, 512], BF16, name="cos_t", tag="cos_t")
                    sin_t = KS.tile([64, 512], BF16, name="sin_t", tag="sin_t")
                    nc.gpsimd.dma_start(cos_t[:], cossinT[0:64, ncols])
                    nc.gpsimd.dma_start(sin_t[:], cossinT[64:128, ncols])
                    kxr = KS.tile([64, 512], BF16, name="kxr", tag="kxr")
                    nc.sync.dma_start(kxr[0:32, :], ckv[4][32:64, :])
                    nc.sync.dma_start(kxr[32:64, :], ckv[4][0:32, :])
                    kt1 = KS.tile([64, 512], BF16, name="kt1", tag="kt1")
                    nc.vector.tensor_mul(kt1[:], ckv[4][:], cos_t[:])
                    nc.vector.tensor_mul(kxr[:], kxr[:], sin_t[:])
                    nc.vector.tensor_add(krot2[0:64, ncols], kt1[:], kxr[:])
                    nc.sync.dma_start(krot2[64:128, ncols],
                                      krot2[0:64, ncols])

                    # rms chain -> scale ckv[0..3] in place
                    ps_ss = PSB.tile([1, 512], F32, name="ps_ssk",
                                     tag="ps_ssk")
                    for m in range(4):
                        sq = KS.tile([128, 512], BF16, name="sqk", tag="sqk")
                        nc.vector.tensor_mul(sq[:], ckv[m][:], ckv[m][:])
                        nc.tensor.matmul(ps_ss[:], ones_b[:], sq[:],
                                         start=(m == 0), stop=(m == 3))
                    srt_k = KS.tile([1, 512], F32, name="srt_k", tag="srt_k")
                    nc.scalar.activation(srt_k[:], ps_ss[:], AF.Sqrt,
                                         bias=eps_sb[:], scale=1.0 / KVR)
                    nc.sync.dma_start(scr_k[:, ncols], srt_k[:])
                    ksT = KS.tile([128, 4], F32, name="ksT", tag="ksT")
                    nc.sync.dma_start(ksT[:], transpose_ap(scr_k, n * 512, 4))
                    krcT = KS.tile([128, 4], BF16, name="krcT", tag="krcT")
                    with nc.allow_low_precision(
                            reason="bf16 rms scales: 0.4% rel err ok"):
                        nc.vector.reciprocal(krcT[:], ksT[:])
                    nc.sync.dma_start(transpose_ap(scr_k2, n * 512, 4),
                                      krcT[:])
                    kscale_bc = KS.tile([128, 512], BF16, name="kscale_bc",
                                        tag="kscale_bc")
                    nc.sync.dma_start(kscale_bc[:],
                                      bcast_src(scr_k2, n * 512, 512))
                    for m in range(4):
                        nc.vector.tensor_mul(ckv[m][:], ckv[m][:],
                                             kscale_bc[:])

                def kv_b(n):
                    ncols = slice(n * 512, (n + 1) * 512)
                    ckv = ckv_n[n]
                    for h in range(KV):
                        ps = PSB.tile([128, 512], F32, name="ps_kn",
                                      tag="ps_kn")
                        for k in range(4):
                            nc.tensor.matmul(
                                ps[:], wkvbk_sb[k][:, h * 128:(h + 1) * 128],
                                ckv[k][:], start=(k == 0), stop=(k == 3))
                        nc.vector.tensor_copy(knopeT[h][:, ncols], ps[:])
                    for mm in range(4):
                        ps = PSB.tile([128, 512], F32, name="ps_v",
                                      tag="ps_v")
                        for k in range(4):
                            nc.tensor.matmul(
                                ps[:], ckv[k][:, mm * 128:(mm + 1) * 128],
                                wkvbv_sb[k][:], start=(k == 0), stop=(k == 3))
                        nc.vector.tensor_copy(v_sb[n * 4 + mm][:], ps[:])

                for n in range(4):
                    kv_a(n)
                    if n > 0:
                        kv_b(n - 1)
                kv_b(3)

            # ========================= Q window =========================
            with (
                tc.tile_pool(name="qwin", bufs=1) as QW,
                tc.tile_pool(name="qsc", bufs=2) as QS,
            ):
                # this core's hidden columns [2048, 256] in 16 chunks
                hq_sb = [QW.tile([128, TPC], BF16, name=f"hq{k}")
                         for k in range(16)]
                for k in range(16):
                    nc.gpsimd.dma_start(hq_sb[k][:],
                                        hTq[k * 128:(k + 1) * 128, :])
                # q_a^T [1536, 256] bf16; rms-scaled in place afterwards
                qaT = [QW.tile([128, TPC], BF16, name=f"qaT{m}")
                       for m in range(12)]
                with tc.tile_pool(name="qaps", bufs=1, space="PSUM") as PSB:
                    for half in range(2):
                        pss = [PSB.tile([128, TPC], F32, name=f"ps_qa{m}",
                                        tag=f"ps_qa{m}") for m in range(6)]
                        for k in range(16):
                            wch = WS.tile([128, 768], BF16, name="wch",
                                          tag="wch")
                            nc.gpsimd.dma_start(
                                wch[:], wqa[k * 128:(k + 1) * 128,
                                            half * 768:(half + 1) * 768])
                            for m in range(6):
                                nc.tensor.matmul(
                                    pss[m][:], wch[:, m * 128:(m + 1) * 128],
                                    hq_sb[k][:], start=(k == 0),
                                    stop=(k == 15))
                        for m in range(6):
                            nc.vector.tensor_copy(qaT[half * 6 + m][:],
                                                  pss[m][:])

                    ps_qss = PSB.tile([1, TPC], F32, name="ps_qss")
                    for m in range(12):
                        sq = QS.tile([128, TPC], BF16, name="sqq", tag="sqq")
                        nc.vector.tensor_mul(sq[:], qaT[m][:], qaT[m][:])
                        nc.tensor.matmul(ps_qss[:], ones_b[:], sq[:],
                                         start=(m == 0), stop=(m == 11))
                    srt_q = QW.tile([1, TPC], F32, name="srt_q")
                    nc.scalar.activation(srt_q[:], ps_qss[:], AF.Sqrt,
                                         bias=eps_sb[:], scale=1.0 / QR)
                    nc.sync.dma_start(scr_q[:], srt_q[:])
                    qsT = QW.tile([128, TPC // 128], F32, name="qsT")
                    nc.sync.dma_start(qsT[:], transpose_ap(scr_q, 0,
                                                           TPC // 128))
                    qrcT = QW.tile([128, TPC // 128], BF16, name="qrcT")
                    with nc.allow_low_precision(
                            reason="bf16 rms scales: 0.4% rel err ok"):
                        nc.vector.reciprocal(qrcT[:], qsT[:])
                    nc.sync.dma_start(transpose_ap(scr_q2, 0, TPC // 128),
                                      qrcT[:])
                    qscale_bc = QW.tile([128, TPC], BF16, name="qscale_bc")
                    nc.sync.dma_start(qscale_bc[:], bcast_src(scr_q2, 0, TPC))
                    for m in range(12):
                        nc.vector.tensor_mul(qaT[m][:], qaT[m][:],
                                             qscale_bc[:])

                # q_b: nope per head + rope pairs (plain evacs; qaT already
                # carries the rms scale)
                cosq_sb = QW.tile([128, TPC], BF16, name="cosq_sb")
                sinq_sb = QW.tile([128, TPC], BF16, name="sinq_sb")
                nc.gpsimd.dma_start(cosq_sb[:], cosq2[:, :])
                nc.gpsimd.dma_start(sinq_sb[:], sinq2[:, :])
                with tc.tile_pool(name="qbps", bufs=1, space="PSUM") as PSB:
                    for g in range(4):
                        psn = [PSB.tile([128, TPC], F32, name=f"ps_qb{u}",
                                        tag=f"ps_qb{u}") for u in range(6)]
                        for k in range(12):
                            wch = WS.tile([128, 768], BF16, name="wch",
                                          tag="wch")
                            nc.gpsimd.dma_start(
                                wch[:, 0:512],
                                wqbn[k * 128:(k + 1) * 128,
                                     g * 512:(g + 1) * 512])
                            nc.gpsimd.dma_start(
                                wch[:, 512:768],
                                wqbr[k * 128:(k + 1) * 128,
                                     g * 256:(g + 1) * 256])
                            for l in range(4):
                                nc.tensor.matmul(
                                    psn[l][:], wch[:, l * 128:(l + 1) * 128],
                                    qaT[k][:], start=(k == 0), stop=(k == 11))
                            for lj in range(2):
                                nc.tensor.matmul(
                                    psn[4 + lj][:],
                                    wch[:, 512 + lj * 128:512 + (lj + 1) * 128],
                                    qaT[k][:], start=(k == 0), stop=(k == 11))
                        for l in range(4):
                            h = g * 4 + l
                            nc.vector.tensor_copy(
                                qn_pair[h // 2][:, h % 2, :], psn[l][:])
                        # RoPE on q pairs (rows 0-63 head 2j, 64-127 head
                        # 2j+1): out = x*cos2 + rot(x)*sin2, rot = partition
                        # rotate by 32 within each 64-row block (sbuf DMA),
                        # rotate_half sign folded into sin2 host-side.
                        # Results go into the zero-padded pair layout.
                        for lj in range(2):
                            j = g * 2 + lj
                            tmp = QS.tile([128, TPC], BF16, name="tmpr",
                                          tag="tmpr")
                            nc.vector.tensor_copy(tmp[:], psn[4 + lj][:])
                            xr = QS.tile([128, TPC], BF16, name="xr", tag="xr")
                            for b0, b1 in ((0, 32), (32, 0), (64, 96),
                                           (96, 64)):
                                nc.sync.dma_start(xr[b0:b0 + 32, :],
                                                  tmp[b1:b1 + 32, :])
                            t1 = QS.tile([128, TPC], BF16, name="t1q",
                                         tag="t1q")
                            nc.vector.tensor_mul(t1[:], tmp[:], cosq_sb[:])
                            nc.vector.tensor_mul(xr[:], xr[:], sinq_sb[:])
                            nc.vector.tensor_add(qr_pair[j][0:64, 0, :],
                                                 t1[0:64, :], xr[0:64, :])
                            nc.vector.tensor_add(qr_pair[j][64:128, 1, :],
                                                 t1[64:128, :], xr[64:128, :])

            # =========================== Attention ==========================
            # Head pairs (2j, 2j+1) share one kv head (hk = j//2) and are
            # processed together: scores psum [128 keys, 2, TPC q-cols].
            # Block kb only touches query cols >= 16*kb (cols below are
            # fully causal-masked); the diagonal band cols 16kb..16kb+16 get
            # the additive band mask (identical for every kb).
            attn_pair = [P.tile([128, 2, TPC], BF16, name=f"attnp{j}")
                         for j in range(NPAIR)]

            with tc.tile_pool(name="aps", bufs=2, space="PSUM") as PSA:
                for j in range(NPAIR):
                    hk = j // 2
                    ps_av = PSA.tile([128, 2, TPC], F32, name="ps_av",
                                     tag="ps_av")
                    ps_sum = PSA.tile([1, 2, TPC], F32, name="ps_sum",
                                      tag="ps_sum")
                    for kb in range(NB):
                        q0 = 16 * kb
                        kcols = slice(kb * 128, (kb + 1) * 128)
                        ps_sc = PSA.tile([128, 2, TPC], F32, name="ps_sc",
                                         tag="ps_sc")
                        nc.tensor.matmul(ps_sc[:, :, q0:],
                                         knopeT[hk][:, kcols],
                                         qn_pair[j][:, :, q0:], start=True,
                                         stop=False)
                        nc.tensor.matmul(ps_sc[:, :, q0:],
                                         krot2[:, kcols],
                                         qr_pair[j][:, :, q0:],
                                         start=False, stop=True)
                        nc.vector.tensor_add(ps_sc[:, :, q0:q0 + 16],
                                             ps_sc[:, :, q0:q0 + 16],
                                             bmask_sb[:])
                        p_t = PP.tile([128, 2, TPC], BF16, name="p_t",
                                      tag="p_t")
                        nc.scalar.activation(p_t[:, :, q0:], ps_sc[:, :, q0:],
                                             AF.Exp, scale=SCALE)
                        nc.tensor.matmul(ps_sum[:, :, q0:], ones_b[:],
                                         p_t[:, :, q0:], start=(kb == 0),
                                         stop=(kb == NB - 1))
                        nc.tensor.matmul(
                            ps_av[:, :, q0:],
                            v_sb[kb][:, hk * 128:(hk + 1) * 128],
                            p_t[:, :, q0:], start=(kb == 0),
                            stop=(kb == NB - 1))
                    # per-pair normalize, overlapped with the next pair's
                    # attention: sums -> DRAM -> transposed [128,4]
                    # reciprocal -> DRAM -> partition broadcast; the evac
                    # multiply folds normalization into one DVE op.
                    s1 = PP.tile([1, 2 * TPC], F32, name="s1", tag="s1",
                                 bufs=3)
                    nc.vector.tensor_copy(
                        s1[:], ps_sum[:].rearrange("o h q -> o (h q)"))
                    nc.sync.dma_start(scr_r[:, j * 2 * TPC:(j + 1) * 2 * TPC],
                                      s1[:])
                    rT = PP.tile([128, 4], F32, name="rT", tag="rT", bufs=3)
                    nc.sync.dma_start(rT[:],
                                      transpose_ap(scr_r, j * 2 * TPC, 4))
                    rrT = PP.tile([128, 4], F32, name="rrT", tag="rrT",
                                  bufs=3)
                    nc.vector.reciprocal(rrT[:], rT[:])
                    nc.sync.dma_start(transpose_ap(scr_r2, j * 2 * TPC, 4),
                                      rrT[:])
                    rb = PP.tile([128, 2, TPC], F32, name="rb", tag="rb",
                                 bufs=2)
                    nc.sync.dma_start(rb[:].rearrange("p h q -> p (h q)"),
                                      bcast_src(scr_r2, j * 2 * TPC, 2 * TPC))
                    nc.vector.tensor_mul(attn_pair[j][:], ps_av[:], rb[:])

            # ============================ o_proj ============================
            with tc.tile_pool(name="ops", bufs=1, space="PSUM") as PSB:
                for n in range(4):
                    ncols = slice(n * 512, (n + 1) * 512)
                    pso = [PSB.tile([128, 512], F32, name=f"ps_o{m}",
                                    tag=f"ps_o{m}") for m in range(2)]
                    for h in range(H):
                        wch = WS.tile([128, 512], BF16, name="woch",
                                      tag="woch", bufs=4)
                        nc.gpsimd.dma_start(wch[:],
                                            wo_t[h * 128:(h + 1) * 128, ncols])
                        for m in range(2):
                            nc.tensor.matmul(
                                pso[m][:],
                                attn_pair[h // 2][:, h % 2,
                                                  m * 128:(m + 1) * 128],
                                wch[:], start=(h == 0),
                                stop=(h == H - 1))
                    for m in range(2):
                        osb = PP.tile([128, 512], F32, name="osb", tag="osb",
                                      bufs=2)
                        nc.vector.tensor_copy(osb[:], pso[m][:])
                        nc.sync.dma_start(out[m * 128:(m + 1) * 128, ncols],
                                          osb[:])

    return nc


def kernel(hidden_states, cos, sin, wq_a, q_a_ln_w, wq_b, wkv_a, kv_a_ln_w,
           wkv_b, wo, cache_position, _trace=False):
    global _BUILT, LAST_RESULTS
    hidden_states = np.asarray(hidden_states, dtype=np.float32)
    cos = np.asarray(cos, dtype=np.float32)
    sin = np.asarray(sin, dtype=np.float32)
    wq_a = np.asarray(wq_a, dtype=np.float32)
    q_a_ln_w = np.asarray(q_a_ln_w, dtype=np.float32)
    wq_b = np.asarray(wq_b, dtype=np.float32)
    wkv_a = np.asarray(wkv_a, dtype=np.float32)
    kv_a_ln_w = np.asarray(kv_a_ln_w, dtype=np.float32)
    wkv_b = np.asarray(wkv_b, dtype=np.float32)
    wo = np.asarray(wo, dtype=np.float32)
    cp = np.asarray(cache_position).astype(np.int64)

    def b16(x):
        return np.ascontiguousarray(np.asarray(x, np.float32).astype(bfloat16))

    # ---- host-side prep (layout/sharding only) ----
    h = hidden_states[0]                       # [S, D]
    hTf = np.ascontiguousarray(h.T)            # [D, S] f32
    hT = b16(hTf)
    cos_sel = cos[0][cp]                       # [S, DR]
    sin_sel = sin[0][cp]
    cosT = np.ascontiguousarray(cos_sel.T)     # [DR, S]
    sinT = np.ascontiguousarray(sin_sel.T)
    # fold the rmsnorm elementwise weights into the b-projections
    wqb_eff = wq_b * q_a_ln_w[:, None]
    wqb_r3 = wqb_eff.reshape(QR, H, DQK)
    wqbn = b16(wqb_r3[:, :, :DN].reshape(QR, H * DN))
    wqbr = b16(wqb_r3[:, :, DN:].reshape(QR, H * DR))
    wkvb_eff = wkv_b * kv_a_ln_w[:, None]      # [KVR, KV*(DN+DV)]
    wkvb_r = wkvb_eff.reshape(KVR, KV, DN + DV)
    wkvbk = b16(wkvb_r[:, :, :DN].reshape(KVR, KV * DN))
    wkvbv = b16(wkvb_r[:, :, DN:].reshape(KVR, KV * DV))
    wo_c = b16(wo)
    wqa_c = b16(wq_a)
    wkva_c = b16(wkv_a)

    sgn = np.concatenate([-np.ones(DR // 2), np.ones(DR // 2)]
                         ).astype(np.float32)[:, None]
    sinS = sinT * sgn
    cossinT = b16(np.concatenate([cosT, sinS], axis=0))
    in_maps = []
    for c in range(NC_):
        toks = np.arange(c, S, NC_)            # this core's 256 query tokens
        hTq = b16(hTf[:, toks])
        cq = cosT[:, toks]
        sq = sinS[:, toks]
        cosq2 = b16(np.concatenate([cq, cq], axis=0))
        sinq2 = b16(np.concatenate([sq, sq], axis=0))
        # band mask bm[k, h, i] = 0 if k <= 8i + c else NEG (same for all
        # key blocks); device layout [k, h*16+i]
        k_ = np.arange(128)[:, None]
        i_ = np.arange(16)[None, :]
        bm = np.where(k_ <= 8 * i_ + c, 0.0, NEG).astype(np.float32)
        bm_dev = np.ascontiguousarray(
            np.repeat(bm[:, None, :], 2, axis=1).reshape(128, 32))
        in_maps.append({
            "hT": hT, "hTq": hTq, "wqa": wqa_c,
            "wqbn": wqbn, "wqbr": wqbr,
            "wkva": wkva_c, "wkvbk": wkvbk, "wkvbv": wkvbv, "wo": wo_c,
            "cossinT": cossinT, "cosq2": cosq2, "sinq2": sinq2,
            "bmask": bm_dev,
        })

    if _BUILT is None:
        _BUILT = _build()
    nc = _BUILT

    res = run_bass_kernel_spmd(nc, in_maps, core_ids=list(range(NC_)),
                               trace=_trace)
    LAST_RESULTS = res

    out_full = np.empty((S, D), dtype=np.float32)
    for c in range(NC_):
        out_full[c::NC_] = res.results[c]["out"]   # row m <-> token 8m+c
    return out_full[None]                      # [1, S, D]


# revision 25
# speedup vs baseline: 1.0691x; 1.0691x over previous
"""DeepseekV3 attention (B=1, S=2048, D=2048, H=16, KV=4) on 8 trn2 cores.

Sharding: token-modulo-8 split. Core c owns query tokens {t : t % 8 == c}
(256 each) — causal attention work is identical on every core, so one SPMD
program serves all 8 cores with per-core DATA (host-sliced hidden columns,
cos/sin slices, causal band masks) carrying the differences.

All matmul operands are bf16 (fp32r runs as fp32_mode=HIGH at 4 cyc/row;
bf16 is 1 cyc/row and halves DMA). Attention processes head PAIRS in single
wide matmuls (rope q zero-padded into complementary partition halves so one
128-contraction matmul serves both heads), skips fully-masked query columns
(block kb only needs cols >= 16*kb), and applies the causal mask only on the
16-column diagonal band (identical [128,16] mask for every kb).

Scheduling: RMS scales are applied to q_a / c_kv ONCE (in place) so the
b-projection evacuations have no cross-engine gating; the kv token chunks
are software-pipelined (chunk n's b-projections run after chunk n+1's
a-projection, hiding the rms-chain latency); reciprocals run full-width on
DRAM-transposed [128, n] layouts; softmax normalization happens per head
pair, overlapped with the next pair's attention, folded into the AV psum
evacuation; bulk weight streams ride the otherwise-idle GpSimd DMA queue
while latency-critical small DMAs keep the Sync queue.
"""
import math
import sys
import types

import numpy as np
from ml_dtypes import bfloat16

# ---------------------------------------------------------------------------
# Container compat: this walrus build rejects instructions carrying more than
# one sync-wait command. Patch Tile to (a) split multi-wait instructions into
# single-wait NoOps on the same engine, (b) hoist the end-of-kernel drain's
# waits onto single-wait NOPs. Also register the NTFF profile hook (the
# image's antenv lacks axon_hooks) so trace=True works for profiling.
# ---------------------------------------------------------------------------
import concourse.bass as bass
import concourse.mybir as mybir
import concourse.tile as tile
from concourse.bass_utils import run_bass_kernel_spmd
from concourse.tile import ScopedClock
from bass_rust import VectorClock

N_PROCS = len(VectorClock())
_PATCHED = False


def _install_ntff_hook():
    if 'antenv.axon_hooks' in sys.modules:
        return
    m = types.ModuleType('antenv.axon_hooks')
    holder = [None]
    m.set_axon_ntff_profile_hook = lambda h: holder.__setitem__(0, h)
    m.get_axon_ntff_profile_hook = lambda: holder[0]
    sys.modules['antenv.axon_hooks'] = m
    try:
        from trn_agent_boot.trn_boot import _ntff_profile_via_ctypes
        m.set_axon_ntff_profile_hook(
            _ntff_profile_via_ctypes('/opt/axon/libaxon_pjrt.so'))
    except Exception:
        pass


def _patched_drain_and_barrier(self, tick_clock, wait_clock):
    gc = tick_clock.global_clock
    for p in range(N_PROCS):
        if gc[p] == 0:
            continue
        single = VectorClock([gc[q] if q == p else 0 for q in range(N_PROCS)])
        nop_inst = self.nc.sync.nop(nofuse=True)
        wait_clock.add_sem_waits(nop_inst.ins, ScopedClock({None: single}))
    self.nc.sync.drain()
    self.nc.all_engine_barrier()
    popped = self.nc._tile_sem_poison_stack.pop()
    assert popped is self._sem_poison
    self.nc.clear_and_free_semaphores(list(self.sems.allocated().values()))
    self.nc.all_engine_barrier()


def _make_split_lower(orig):
    def _split_multi_waits(self, ordered):
        nc = self.nc
        for bb_name, insts in ordered.items():
            out = []
            for inst in insts:
                si = inst.sync_info
                waits = list(si.on_wait) if si is not None else []
                if len(waits) > 1:
                    for w in waits[:-1]:
                        nop = mybir.InstNoOp(
                            name=f"{inst.name}-waitsplit-{nc.next_id()}",
                            engine=inst.engine,
                            sync_info=mybir.SyncInfo(on_wait=[w], on_update=[]),
                        )
                        nc.register_instruction(nop)
                        out.append(nop)
                    inst.sync_info = mybir.SyncInfo(
                        on_wait=[waits[-1]], on_update=list(si.on_update))
                out.append(inst)
            ordered[bb_name] = out
        return orig(self, ordered)
    return _split_multi_waits


def _install_patches():
    global _PATCHED
    _install_ntff_hook()
    if _PATCHED:
        return
    tile.TileContext._drain_and_barrier = _patched_drain_and_barrier
    tile.TileContext._lower_ordered_insts = _make_split_lower(
        tile.TileContext._lower_ordered_insts)
    _PATCHED = True


_install_patches()

# ---------------------------------------------------------------------------
# Problem constants (hardcoded per the spec).
# ---------------------------------------------------------------------------
S = 2048
D = 2048
H = 16
KV = 4
GROUPS = H // KV
DN = 128          # d_nope
DR = 64           # d_rope
DQK = DN + DR     # 192
DV = 128
QR = 1536         # q rank
KVR = 512         # kv rank
EPS = 1e-6
NC_ = 8           # cores
TPC = S // NC_    # 256 tokens per core
NB = S // 128     # 16 k-subtiles
NPAIR = H // 2    # 8 head pairs
SCALE = 1.0 / math.sqrt(DQK)
NEG = -1e30

F32 = mybir.dt.float32
BF16 = mybir.dt.bfloat16
AF = mybir.ActivationFunctionType

_BUILT = None     # cached (nc,) so repeat kernel() calls skip rebuild
LAST_RESULTS = None  # BassKernelResults stash for test.py


def _build():
    nc = bass.Bass()

    # ---- DRAM I/O (identical declaration on all cores; data differs) ----
    hT = nc.dram_tensor("hT", [D, S], BF16, kind="ExternalInput")
    hTq = nc.dram_tensor("hTq", [D, TPC], BF16, kind="ExternalInput")
    wqa = nc.dram_tensor("wqa", [D, QR], BF16, kind="ExternalInput")
    wqbn = nc.dram_tensor("wqbn", [QR, H * DN], BF16, kind="ExternalInput")
    wqbr = nc.dram_tensor("wqbr", [QR, H * DR], BF16, kind="ExternalInput")
    wkva = nc.dram_tensor("wkva", [D, KVR + DR], BF16, kind="ExternalInput")
    wkvbk = nc.dram_tensor("wkvbk", [KVR, KV * DN], BF16, kind="ExternalInput")
    wkvbv = nc.dram_tensor("wkvbv", [KVR, KV * DV], BF16, kind="ExternalInput")
    wo_t = nc.dram_tensor("wo", [H * DV, D], BF16, kind="ExternalInput")
    cossinT = nc.dram_tensor("cossinT", [2 * DR, S], BF16,
                             kind="ExternalInput")
    cosq2 = nc.dram_tensor("cosq2", [2 * DR, TPC], BF16, kind="ExternalInput")
    sinq2 = nc.dram_tensor("sinq2", [2 * DR, TPC], BF16, kind="ExternalInput")
    bmask = nc.dram_tensor("bmask", [128, 2 * 16], F32, kind="ExternalInput")
    out = nc.dram_tensor("out", [TPC, D], F32, kind="ExternalOutput")
    # scratch for free->partition broadcasts / transposed reciprocals
    scr_k = nc.dram_tensor("scr_k", [1, S], F32, kind="Internal")
    scr_k2 = nc.dram_tensor("scr_k2", [1, S], BF16, kind="Internal")
    scr_q = nc.dram_tensor("scr_q", [1, TPC], F32, kind="Internal")
    scr_q2 = nc.dram_tensor("scr_q2", [1, TPC], BF16, kind="Internal")
    scr_r = nc.dram_tensor("scr_r", [1, NPAIR * 2 * TPC], F32, kind="Internal")
    scr_r2 = nc.dram_tensor("scr_r2", [1, NPAIR * 2 * TPC], F32,
                            kind="Internal")

    def bcast_src(dram, off, ncols):
        # element off.. of the flat DRAM vector, broadcast to 128 partitions
        ap = dram[:]
        return bass.AP(tensor=ap.tensor, offset=ap.offset + off,
                       ap=[[0, 128], [1, ncols]])

    def transpose_ap(dram, off, nfree):
        # [128, nfree] view of dram[off .. off+128*nfree) with partition
        # stride 1 (transposed load/store for 1-row vectors)
        ap = dram[:]
        return bass.AP(tensor=ap.tensor, offset=ap.offset + off,
                       ap=[[1, 128], [128, nfree]])

    with tile.TileContext(nc) as tc:
        with (
            tc.tile_pool(name="persist", bufs=1) as P,   # attention-lived
            tc.tile_pool(name="wstream", bufs=3) as WS,  # streamed weights
            tc.tile_pool(name="ppool", bufs=6) as PP,    # p tiles (bf16)
        ):
            ones_b = P.tile([128, 1], BF16, name="ones_b")
            nc.vector.memset(ones_b[:], 1.0)
            eps_sb = P.tile([1, 1], F32, name="eps_sb")
            nc.vector.memset(eps_sb[:], EPS)
            # band mask [k, head, i]: 0 if k <= 8i + c else NEG (same for
            # every key block kb; applies to query cols 16kb..16kb+16)
            bmask_sb = P.tile([128, 2, 16], F32, name="bmask_sb")
            nc.sync.dma_start(bmask_sb[:], bmask[:, :].rearrange(
                "k (h i) -> k h i", h=2))

            # attention-lived products, head-pair packed
            qn_pair = [P.tile([128, 2, TPC], BF16, name=f"qnp{j}")
                       for j in range(NPAIR)]
            qr_pair = [P.tile([128, 2, TPC], BF16, name=f"qrp{j}")
                       for j in range(NPAIR)]
            for j in range(NPAIR):
                nc.vector.memset(qr_pair[j][:], 0.0)
            knopeT = [P.tile([128, S], BF16, name=f"knopeT{h}")
                      for h in range(KV)]
            v_sb = [P.tile([128, KV * DV], BF16, name=f"v{m}")
                    for m in range(16)]
            # k_rot^T duplicated in both partition halves so the rope scores
            # matmul can serve both heads of a pair in one 128-contraction
            krot2 = P.tile([128, S], BF16, name="krot2")

            # ========================= KV window =========================
            # replicated over all 2048 tokens, 4 chunks of 512, software
            # pipelined: chunk n's b-projections are issued after chunk
            # n+1's a-projection so the rms chain latency hides under PE
            # work. ckv is rms-scaled IN PLACE once per chunk, so the
            # b-projection evacuations are plain copies with no gating.
            with (
                tc.tile_pool(name="kvwin", bufs=1) as KW,
                tc.tile_pool(name="kvch", bufs=2) as KC,
                tc.tile_pool(name="ksc", bufs=2) as KS,
                tc.tile_pool(name="kps", bufs=1, space="PSUM") as PSB,
            ):
                wkva_sb = [KW.tile([128, KVR + DR], BF16, name=f"wkva{k}")
                           for k in range(16)]
                for k in range(16):
                    nc.scalar.dma_start(wkva_sb[k][:],
                                        wkva[k * 128:(k + 1) * 128, :])
                wkvbk_sb = [KW.tile([128, KV * DN], BF16, name=f"wkvbk{k}")
                            for k in range(4)]
                wkvbv_sb = [KW.tile([128, KV * DV], BF16, name=f"wkvbv{k}")
                            for k in range(4)]
                for k in range(4):
                    nc.scalar.dma_start(wkvbk_sb[k][:],
                                        wkvbk[k * 128:(k + 1) * 128, :])
                    nc.scalar.dma_start(wkvbv_sb[k][:],
                                        wkvbv[k * 128:(k + 1) * 128, :])

                m_sizes = [128, 128, 128, 128, 64]
                ckv_n = [None] * 4   # per-chunk scaled ckv tiles

                def kv_a(n):
                    ncols = slice(n * 512, (n + 1) * 512)
                    ckv = [KC.tile([m_sizes[m], 512], BF16, name=f"ckv{m}",
                                   tag=f"ckv{m}") for m in range(5)]
                    ckv_n[n] = ckv
                    pss = [PSB.tile([m_sizes[m], 512], F32, name=f"ps_kva{m}",
                                    tag=f"ps_kva{m}") for m in range(5)]
                    for k in range(16):
                        hch = WS.tile([128, 512], BF16, name="hch", tag="hch")
                        nc.scalar.dma_start(hch[:],
                                            hT[k * 128:(k + 1) * 128, ncols])
                        for m in range(5):
                            nc.tensor.matmul(
                                pss[m][:],
                                wkva_sb[k][:, m * 128: m * 128 + m_sizes[m]],
                                hch[:], start=(k == 0), stop=(k == 15))
                    for m in range(5):
                        nc.vector.tensor_copy(ckv[m][:], pss[m][:])

                    # RoPE on k_rot (raw; no rms on the rope part)
                    cos_t = KS.tile([64, 512], BF16, name="cos_t", tag="cos_t")
                    sin_t = KS.tile([64, 512], BF16, name="sin_t", tag="sin_t")
                    nc.scalar.dma_start(cos_t[:], cossinT[0:64, ncols])
                    nc.scalar.dma_start(sin_t[:], cossinT[64:128, ncols])
                    kxr = KS.tile([64, 512], BF16, name="kxr", tag="kxr")
                    nc.sync.dma_start(kxr[0:32, :], ckv[4][32:64, :])
                    nc.sync.dma_start(kxr[32:64, :], ckv[4][0:32, :])
                    kt1 = KS.tile([64, 512], BF16, name="kt1", tag="kt1")
                    nc.vector.tensor_mul(kt1[:], ckv[4][:], cos_t[:])
                    nc.vector.tensor_mul(kxr[:], kxr[:], sin_t[:])
                    nc.vector.tensor_add(krot2[0:64, ncols], kt1[:], kxr[:])
                    nc.sync.dma_start(krot2[64:128, ncols],
                                      krot2[0:64, ncols])

                    # rms chain -> scale ckv[0..3] in place
                    ps_ss = PSB.tile([1, 512], F32, name="ps_ssk",
                                     tag="ps_ssk")
                    for m in range(4):
                        sq = KS.tile([128, 512], BF16, name="sqk", tag="sqk")
                        nc.vector.tensor_mul(sq[:], ckv[m][:], ckv[m][:])
                        nc.tensor.matmul(ps_ss[:], ones_b[:], sq[:],
                                         start=(m == 0), stop=(m == 3))
                    srt_k = KS.tile([1, 512], F32, name="srt_k", tag="srt_k")
                    nc.scalar.activation(srt_k[:], ps_ss[:], AF.Sqrt,
                                         bias=eps_sb[:], scale=1.0 / KVR)
                    nc.sync.dma_start(scr_k[:, ncols], srt_k[:])
                    ksT = KS.tile([128, 4], F32, name="ksT", tag="ksT")
                    nc.sync.dma_start(ksT[:], transpose_ap(scr_k, n * 512, 4))
                    krcT = KS.tile([128, 4], BF16, name="krcT", tag="krcT")
                    with nc.allow_low_precision(
                            reason="bf16 rms scales: 0.4% rel err ok"):
                        nc.vector.reciprocal(krcT[:], ksT[:])
                    nc.sync.dma_start(transpose_ap(scr_k2, n * 512, 4),
                                      krcT[:])
                    kscale_bc = KS.tile([128, 512], BF16, name="kscale_bc",
                                        tag="kscale_bc")
                    nc.sync.dma_start(kscale_bc[:],
                                      bcast_src(scr_k2, n * 512, 512))
                    for m in range(4):
                        nc.vector.tensor_mul(ckv[m][:], ckv[m][:],
                                             kscale_bc[:])

                def kv_b(n):
                    ncols = slice(n * 512, (n + 1) * 512)
                    ckv = ckv_n[n]
                    for h in range(KV):
                        ps = PSB.tile([128, 512], F32, name="ps_kn",
                                      tag="ps_kn")
                        for k in range(4):
                            nc.tensor.matmul(
                                ps[:], wkvbk_sb[k][:, h * 128:(h + 1) * 128],
                                ckv[k][:], start=(k == 0), stop=(k == 3))
                        nc.vector.tensor_copy(knopeT[h][:, ncols], ps[:])
                    for mm in range(4):
                        ps = PSB.tile([128, 512], F32, name="ps_v",
                                      tag="ps_v")
                        for k in range(4):
                            nc.tensor.matmul(
                                ps[:], ckv[k][:, mm * 128:(mm + 1) * 128],
                                wkvbv_sb[k][:], start=(k == 0), stop=(k == 3))
                        nc.vector.tensor_copy(v_sb[n * 4 + mm][:], ps[:])

                for n in range(4):
                    kv_a(n)
                    if n > 0:
                        kv_b(n - 1)
                kv_b(3)

            # ========================= Q window =========================
            with (
                tc.tile_pool(name="qwin", bufs=1) as QW,
                tc.tile_pool(name="qsc", bufs=2) as QS,
            ):
                # this core's hidden columns [2048, 256] in 16 chunks
                hq_sb = [QW.tile([128, TPC], BF16, name=f"hq{k}")
                         for k in range(16)]
                for k in range(16):
                    nc.scalar.dma_start(hq_sb[k][:],
                                        hTq[k * 128:(k + 1) * 128, :])
                # q_a^T [1536, 256] bf16; rms-scaled in place afterwards
                qaT = [QW.tile([128, TPC], BF16, name=f"qaT{m}")
                       for m in range(12)]
                with tc.tile_pool(name="qaps", bufs=1, space="PSUM") as PSB:
                    for half in range(2):
                        pss = [PSB.tile([128, TPC], F32, name=f"ps_qa{m}",
                                        tag=f"ps_qa{m}") for m in range(6)]
                        for k in range(16):
                            wch = WS.tile([128, 768], BF16, name="wch",
                                          tag="wch")
                            nc.scalar.dma_start(
                                wch[:], wqa[k * 128:(k + 1) * 128,
                                            half * 768:(half + 1) * 768])
                            for m in range(6):
                                nc.tensor.matmul(
                                    pss[m][:], wch[:, m * 128:(m + 1) * 128],
                                    hq_sb[k][:], start=(k == 0),
                                    stop=(k == 15))
                        for m in range(6):
                            nc.vector.tensor_copy(qaT[half * 6 + m][:],
                                                  pss[m][:])

                    ps_qss = PSB.tile([1, TPC], F32, name="ps_qss")
                    for m in range(12):
                        sq = QS.tile([128, TPC], BF16, name="sqq", tag="sqq")
                        nc.vector.tensor_mul(sq[:], qaT[m][:], qaT[m][:])
                        nc.tensor.matmul(ps_qss[:], ones_b[:], sq[:],
                                         start=(m == 0), stop=(m == 11))
                    srt_q = QW.tile([1, TPC], F32, name="srt_q")
                    nc.scalar.activation(srt_q[:], ps_qss[:], AF.Sqrt,
                                         bias=eps_sb[:], scale=1.0 / QR)
                    nc.sync.dma_start(scr_q[:], srt_q[:])
                    qsT = QW.tile([128, TPC // 128], F32, name="qsT")
                    nc.sync.dma_start(qsT[:], transpose_ap(scr_q, 0,
                                                           TPC // 128))
                    qrcT = QW.tile([128, TPC // 128], BF16, name="qrcT")
                    with nc.allow_low_precision(
                            reason="bf16 rms scales: 0.4% rel err ok"):
                        nc.vector.reciprocal(qrcT[:], qsT[:])
                    nc.sync.dma_start(transpose_ap(scr_q2, 0, TPC // 128),
                                      qrcT[:])
                    qscale_bc = QW.tile([128, TPC], BF16, name="qscale_bc")
                    nc.sync.dma_start(qscale_bc[:], bcast_src(scr_q2, 0, TPC))
                    for m in range(12):
                        nc.vector.tensor_mul(qaT[m][:], qaT[m][:],
                                             qscale_bc[:])

                # q_b: nope per head + rope pairs (plain evacs; qaT already
                # carries the rms scale)
                cosq_sb = QW.tile([128, TPC], BF16, name="cosq_sb")
                sinq_sb = QW.tile([128, TPC], BF16, name="sinq_sb")
                nc.scalar.dma_start(cosq_sb[:], cosq2[:, :])
                nc.scalar.dma_start(sinq_sb[:], sinq2[:, :])
                with tc.tile_pool(name="qbps", bufs=1, space="PSUM") as PSB:
                    for g in range(4):
                        psn = [PSB.tile([128, TPC], F32, name=f"ps_qb{u}",
                                        tag=f"ps_qb{u}") for u in range(6)]
                        for k in range(12):
                            wch = WS.tile([128, 768], BF16, name="wch",
                                          tag="wch")
                            nc.scalar.dma_start(
                                wch[:, 0:512],
                                wqbn[k * 128:(k + 1) * 128,
                                     g * 512:(g + 1) * 512])
                            nc.scalar.dma_start(
                                wch[:, 512:768],
                                wqbr[k * 128:(k + 1) * 128,
                                     g * 256:(g + 1) * 256])
                            for l in range(4):
                                nc.tensor.matmul(
                                    psn[l][:], wch[:, l * 128:(l + 1) * 128],
                                    qaT[k][:], start=(k == 0), stop=(k == 11))
                            for lj in range(2):
                                nc.tensor.matmul(
                                    psn[4 + lj][:],
                                    wch[:, 512 + lj * 128:512 + (lj + 1) * 128],
                                    qaT[k][:], start=(k == 0), stop=(k == 11))
                        for l in range(4):
                            h = g * 4 + l
                            nc.vector.tensor_copy(
                                qn_pair[h // 2][:, h % 2, :], psn[l][:])
                        # RoPE on q pairs (rows 0-63 head 2j, 64-127 head
                        # 2j+1): out = x*cos2 + rot(x)*sin2, rot = partition
                        # rotate by 32 within each 64-row block (sbuf DMA),
                        # rotate_half sign folded into sin2 host-side.
                        # Results go into the zero-padded pair layout.
                        for lj in range(2):
                            j = g * 2 + lj
                            tmp = QS.tile([128, TPC], BF16, name="tmpr",
                                          tag="tmpr")
                            nc.vector.tensor_copy(tmp[:], psn[4 + lj][:])
                            xr = QS.tile([128, TPC], BF16, name="xr", tag="xr")
                            for b0, b1 in ((0, 32), (32, 0), (64, 96),
                                           (96, 64)):
                                nc.sync.dma_start(xr[b0:b0 + 32, :],
                                                  tmp[b1:b1 + 32, :])
                            t1 = QS.tile([128, TPC], BF16, name="t1q",
                                         tag="t1q")
                            nc.vector.tensor_mul(t1[:], tmp[:], cosq_sb[:])
                            nc.vector.tensor_mul(xr[:], xr[:], sinq_sb[:])
                            nc.vector.tensor_add(qr_pair[j][0:64, 0, :],
                                                 t1[0:64, :], xr[0:64, :])
                            nc.vector.tensor_add(qr_pair[j][64:128, 1, :],
                                                 t1[64:128, :], xr[64:128, :])

            # =========================== Attention ==========================
            # Head pairs (2j, 2j+1) share one kv head (hk = j//2) and are
            # processed together: scores psum [128 keys, 2, TPC q-cols].
            # Block kb only touches query cols >= 16*kb (cols below are
            # fully causal-masked); the diagonal band cols 16kb..16kb+16 get
            # the additive band mask (identical for every kb).
            attn_pair = [P.tile([128, 2, TPC], BF16, name=f"attnp{j}")
                         for j in range(NPAIR)]

            with tc.tile_pool(name="aps", bufs=2, space="PSUM") as PSA:
                for j in range(NPAIR):
                    hk = j // 2
                    ps_av = PSA.tile([128, 2, TPC], F32, name="ps_av",
                                     tag="ps_av")
                    ps_sum = PSA.tile([1, 2, TPC], F32, name="ps_sum",
                                      tag="ps_sum")
                    for kb in range(NB):
                        q0 = 16 * kb
                        kcols = slice(kb * 128, (kb + 1) * 128)
                        ps_sc = PSA.tile([128, 2, TPC], F32, name="ps_sc",
                                         tag="ps_sc", bufs=3)
                        nc.tensor.matmul(ps_sc[:, :, q0:],
                                         knopeT[hk][:, kcols],
                                         qn_pair[j][:, :, q0:], start=True,
                                         stop=False)
                        nc.tensor.matmul(ps_sc[:, :, q0:],
                                         krot2[:, kcols],
                                         qr_pair[j][:, :, q0:],
                                         start=False, stop=True)
                        nc.vector.tensor_add(ps_sc[:, :, q0:q0 + 16],
                                             ps_sc[:, :, q0:q0 + 16],
                                             bmask_sb[:])
                        p_t = PP.tile([128, 2, TPC], BF16, name="p_t",
                                      tag="p_t")
                        nc.scalar.activation(p_t[:, :, q0:], ps_sc[:, :, q0:],
                                             AF.Exp, scale=SCALE)
                        nc.tensor.matmul(ps_sum[:, :, q0:], ones_b[:],
                                         p_t[:, :, q0:], start=(kb == 0),
                                         stop=(kb == NB - 1))
                        nc.tensor.matmul(
                            ps_av[:, :, q0:],
                            v_sb[kb][:, hk * 128:(hk + 1) * 128],
                            p_t[:, :, q0:], start=(kb == 0),
                            stop=(kb == NB - 1))
                    # per-pair normalize, overlapped with the next pair's
                    # attention: sums -> DRAM -> transposed [128,4]
                    # reciprocal -> DRAM -> partition broadcast; the evac
                    # multiply folds normalization into one DVE op.
                    s1 = PP.tile([1, 2 * TPC], F32, name="s1", tag="s1",
                                 bufs=3)
                    nc.vector.tensor_copy(
                        s1[:], ps_sum[:].rearrange("o h q -> o (h q)"))
                    nc.sync.dma_start(scr_r[:, j * 2 * TPC:(j + 1) * 2 * TPC],
                                      s1[:])
                    rT = PP.tile([128, 4], F32, name="rT", tag="rT", bufs=3)
                    nc.sync.dma_start(rT[:],
                                      transpose_ap(scr_r, j * 2 * TPC, 4))
                    rrT = PP.tile([128, 4], F32, name="rrT", tag="rrT",
                                  bufs=3)
                    nc.vector.reciprocal(rrT[:], rT[:])
                    nc.sync.dma_start(transpose_ap(scr_r2, j * 2 * TPC, 4),
                                      rrT[:])
                    rb = PP.tile([128, 2, TPC], F32, name="rb", tag="rb",
                                 bufs=2)
                    nc.sync.dma_start(rb[:].rearrange("p h q -> p (h q)"),
                                      bcast_src(scr_r2, j * 2 * TPC, 2 * TPC))
                    nc.vector.tensor_mul(attn_pair[j][:], ps_av[:], rb[:])

            # ============================ o_proj ============================
            with tc.tile_pool(name="ops", bufs=1, space="PSUM") as PSB:
                for n in range(4):
                    ncols = slice(n * 512, (n + 1) * 512)
                    pso = [PSB.tile([128, 512], F32, name=f"ps_o{m}",
                                    tag=f"ps_o{m}") for m in range(2)]
                    for h in range(H):
                        wch = WS.tile([128, 512], BF16, name="woch",
                                      tag="woch", bufs=4)
                        nc.sync.dma_start(wch[:],
                                          wo_t[h * 128:(h + 1) * 128, ncols])
                        for m in range(2):
                            nc.tensor.matmul(
                                pso[m][:],
                                attn_pair[h // 2][:, h % 2,
                                                  m * 128:(m + 1) * 128],
                                wch[:], start=(h == 0),
                                stop=(h == H - 1))
                    for m in range(2):
                        osb = PP.tile([128, 512], F32, name="osb", tag="osb",
                                      bufs=2)
                        nc.vector.tensor_copy(osb[:], pso[m][:])
                        nc.sync.dma_start(out[m * 128:(m + 1) * 128, ncols],
                                          osb[:])

    return nc


def kernel(hidden_states, cos, sin, wq_a, q_a_ln_w, wq_b, wkv_a, kv_a_ln_w,
           wkv_b, wo, cache_position, _trace=False):
    global _BUILT, LAST_RESULTS
    hidden_states = np.asarray(hidden_states, dtype=np.float32)
    cos = np.asarray(cos, dtype=np.float32)
    sin = np.asarray(sin, dtype=np.float32)
    wq_a = np.asarray(wq_a, dtype=np.float32)
    q_a_ln_w = np.asarray(q_a_ln_w, dtype=np.float32)
    wq_b = np.asarray(wq_b, dtype=np.float32)
    wkv_a = np.asarray(wkv_a, dtype=np.float32)
    kv_a_ln_w = np.asarray(kv_a_ln_w, dtype=np.float32)
    wkv_b = np.asarray(wkv_b, dtype=np.float32)
    wo = np.asarray(wo, dtype=np.float32)
    cp = np.asarray(cache_position).astype(np.int64)

    def b16(x):
        return np.ascontiguousarray(np.asarray(x, np.float32).astype(bfloat16))

    # ---- host-side prep (layout/sharding only) ----
    h = hidden_states[0]                       # [S, D]
    hTf = np.ascontiguousarray(h.T)            # [D, S] f32
    hT = b16(hTf)
    cos_sel = cos[0][cp]                       # [S, DR]
    sin_sel = sin[0][cp]
    cosT = np.ascontiguousarray(cos_sel.T)     # [DR, S]
    sinT = np.ascontiguousarray(sin_sel.T)
    # fold the rmsnorm elementwise weights into the b-projections
    wqb_eff = wq_b * q_a_ln_w[:, None]
    wqb_r3 = wqb_eff.reshape(QR, H, DQK)
    wqbn = b16(wqb_r3[:, :, :DN].reshape(QR, H * DN))
    wqbr = b16(wqb_r3[:, :, DN:].reshape(QR, H * DR))
    wkvb_eff = wkv_b * kv_a_ln_w[:, None]      # [KVR, KV*(DN+DV)]
    wkvb_r = wkvb_eff.reshape(KVR, KV, DN + DV)
    wkvbk = b16(wkvb_r[:, :, :DN].reshape(KVR, KV * DN))
    wkvbv = b16(wkvb_r[:, :, DN:].reshape(KVR, KV * DV))
    wo_c = b16(wo)
    wqa_c = b16(wq_a)
    wkva_c = b16(wkv_a)

    sgn = np.concatenate([-np.ones(DR // 2), np.ones(DR // 2)]
                         ).astype(np.float32)[:, None]
    sinS = sinT * sgn
    cossinT = b16(np.concatenate([cosT, sinS], axis=0))
    in_maps = []
    for c in range(NC_):
        toks = np.arange(c, S, NC_)            # this core's 256 query tokens
        hTq = b16(hTf[:, toks])
        cq = cosT[:, toks]
        sq = sinS[:, toks]
        cosq2 = b16(np.concatenate([cq, cq], axis=0))
        sinq2 = b16(np.concatenate([sq, sq], axis=0))
        # band mask bm[k, h, i] = 0 if k <= 8i + c else NEG (same for all
        # key blocks); device layout [k, h*16+i]
        k_ = np.arange(128)[:, None]
        i_ = np.arange(16)[None, :]
        bm = np.where(k_ <= 8 * i_ + c, 0.0, NEG).astype(np.float32)
        bm_dev = np.ascontiguousarray(
            np.repeat(bm[:, None, :], 2, axis=1).reshape(128, 32))
        in_maps.append({
            "hT": hT, "hTq": hTq, "wqa": wqa_c,
            "wqbn": wqbn, "wqbr": wqbr,
            "wkva": wkva_c, "wkvbk": wkvbk, "wkvbv": wkvbv, "wo": wo_c,
            "cossinT": cossinT, "cosq2": cosq2, "sinq2": sinq2,
            "bmask": bm_dev,
        })

    if _BUILT is None:
        _BUILT = _build()
    nc = _BUILT

    res = run_bass_kernel_spmd(nc, in_maps, core_ids=list(range(NC_)),
                               trace=_trace)
    LAST_RESULTS = res

    out_full = np.empty((S, D), dtype=np.float32)
    for c in range(NC_):
        out_full[c::NC_] = res.results[c]["out"]   # row m <-> token 8m+c
    return out_full[None]                      # [1, S, D]
